# revision 1
# baseline (speedup 1.0000x reference)
"""Transformer block kernel for TRN2 (Bass/Tile), one batch element per core.

Computes (per core, x [1024, 768] f32):
    h  = LN(x) (gamma/beta pre-folded into weights on host)
    qk = h @ qkw + qkb ; q = qk[:, :768], k = qk[:, 768:]  (head-major 12x64)
    v  = h @ vw                 (v bias folded into proj bias on host)
    S^T[m,n] = (k_m . q_n) / 8 ;  P = exp(S^T)   (no max subtraction; scores are small)
    oe = [v; 1]^T @ P  -> rows 0..63 = unnormalized o^T, row 64 = softmax denom
    o^T = oe[0:64] / denom
    x1 = x + o @ pw + pb
    h2 = LN2(x1) (folded)
    out = x1 + gelu(h2 @ f1w + f1b) @ f2w + f2b

Layout convention: "feature-major" tensors are [feat_on_partitions, tokens] SBUF
tiles; token-major are [tokens_on_partitions, feat]. LN / residual are
token-major; matmuls contract over partitions so projections run feature-major.
"""

import sys
from contextlib import ExitStack

if "/opt/trn_rl_repo" not in sys.path:
    sys.path.insert(0, "/opt/trn_rl_repo")

import numpy as np

import concourse.bass as bass
import concourse.mybir as mybir
from concourse.masks import make_identity

F32 = mybir.dt.float32
F32R = mybir.dt.float32r
BF16 = mybir.dt.bfloat16
AF = mybir.ActivationFunctionType
ALU = mybir.AluOpType

P = 128
EMB = 768
SEQ = 1024
NH = 12
HD = 64
MLPD = 3072
EC = EMB // P      # 6 embedding chunks
NT = SEQ // P      # 8 token tiles
NC2 = SEQ // 512   # 2 token n-chunks
HC = MLPD // P     # 24 hidden chunks
HP = NH // 2       # 6 head pairs
EPS = 1e-5
SCALE = HD ** -0.5


def r32(ap):
    """Identity; matmul operands are declared float32r at allocation."""
    return ap


def _ln_stats(nc, x_ap, mv, stats, eps_t):
    """bn stats + rstd for one [128, EMB] tile; mv = [mean, rstd]."""
    xg = x_ap.rearrange("p (g d) -> p g d", d=256)
    for g in range(3):
        nc.vector.bn_stats(out=stats[:, g, :], in_=xg[:, g, :])
    nc.vector.bn_aggr(out=mv, in_=stats)
    # rstd = 1/sqrt(var + eps); Sqrt on ACT (one table set), exact recip on DVE
    # ([128,1] is one element per lane - fast)
    nc.scalar.activation(out=mv[:, 1:2], in_=mv[:, 1:2], func=AF.Sqrt, bias=eps_t, scale=1.0)
    nc.vector.reciprocal(out=mv[:, 1:2], in_=mv[:, 1:2])


def _ln_apply(nc, x_ap, h_out, mv):
    nc.vector.tensor_scalar(
        out=h_out,
        in0=x_ap,
        scalar1=mv[:, 0:1],
        scalar2=mv[:, 1:2],
        op0=ALU.subtract,
        op1=ALU.mult,
    )


def _transpose_to_featmajor(nc, tc, pool_ps, pool_sb, src_tok, dstT, t):
    """PE-transpose token-major src_tok [128, EMB] into dstT [:, e, t*128:(t+1)*128]."""
    ident = tc._block_ident
    for group_start, group_n in ((0, 4), (4, 2)):
        ptr = pool_ps.tile([P, 4 * P], BF16, tag="tr", name=f"ptr_t{t}_{group_start}")
        for j in range(group_n):
            e = group_start + j
            nc.tensor.transpose(
                ptr[:, j * P:(j + 1) * P],
                src_tok[:, e * P:(e + 1) * P],
                ident,
            )
        nc.scalar.copy(
            out=dstT[:, group_start:group_start + group_n, t * P:(t + 1) * P],
            in_=ptr[:, :group_n * P].rearrange("p (j q) -> p j q", q=P),
        )


def build_block(tc, outs, ins):
    nc = tc.nc
    x_d = ins["x"]
    qkw_d, qkb_d = ins["qkw"], ins["qkb"]
    vw_d = ins["vw"]
    pw_d, pb_d = ins["pw"], ins["pb"]
    f1w_d, f1b_d = ins["f1w"], ins["f1b"]
    f2w_d, f2b_d = ins["f2w"], ins["f2b"]
    out_d = outs["out"]

    with ExitStack() as ctx:
        consts = ctx.enter_context(tc.tile_pool(name="consts", bufs=1))
        ident = consts.tile([P, P], BF16)
        make_identity(nc, ident)
        tc._block_ident = ident
        eps_t = consts.tile([P, 1], F32)
        nc.vector.memset(eps_t, EPS)
        qkb_sb = consts.tile([P, 2 * EC], F32)
        pb_sb = consts.tile([P, EC], F32)
        f1b_sb = consts.tile([P, HC], F32)
        f2b_sb = consts.tile([P, EC], F32)

        # Persistent SBUF tensors
        glob = ctx.enter_context(tc.tile_pool(name="glob", bufs=1))
        x1 = glob.tile([P, NT, EMB], F32)            # residual stream (starts as x)
        actT = glob.tile([P, EC, SEQ], BF16, tag="actT")  # hT, later h2T reuses slot

        attn_glob = ctx.enter_context(tc.tile_pool(name="attn_glob", bufs=1))
        vext = attn_glob.tile([P, NT, NH, HD + 1], BF16)
        oT = attn_glob.tile([P, EC, SEQ], BF16)      # attention out, feature-major
        vw_sb = attn_glob.tile([P, EC, EMB], BF16)
        pw_sb = attn_glob.tile([P, EC, EMB], BF16)

        work = ctx.enter_context(tc.tile_pool(name="work", bufs=3))
        stat_pool = ctx.enter_context(tc.tile_pool(name="stat", bufs=4))

        # ---- load x into x1 (x1 is BOTH the LN1 input and the residual acc) ----
        x_r = x_d.rearrange("(t p) e -> p t e", p=P)
        for t in range(NT):
            nc.sync.dma_start(out=x1[:, t, :], in_=x_r[:, t, :])

        # ================= Phase A: LN1 + transpose to hT =================
        with tc.tile_pool(name="psA", space="PSUM", bufs=2) as psA:
            hs, mvs = [], []
            for t in range(NT):
                mv = stat_pool.tile([P, 2], F32, tag="mv", bufs=NT, name=f"mv1_{t}")
                stats = stat_pool.tile([P, 3, 6], F32, tag="stats", name=f"st1_{t}")
                _ln_stats(nc, x1[:, t, :], mv, stats, eps_t)
                mvs.append(mv)
            for t in range(NT):
                h_t = work.tile([P, EMB], BF16, tag="h", bufs=NT, name=f"h_{t}")
                _ln_apply(nc, x1[:, t, :], h_t, mvs[t])
                hs.append(h_t)
            for t in range(NT):
                _transpose_to_featmajor(nc, tc, psA, work, hs[t], actT, t)

        # weights / biases (emitted after x+LN so the x DMAs win the queues)
        nc.sync.dma_start(out=vw_sb, in_=vw_d.rearrange("(kc p) o -> p kc o", p=P))
        nc.sync.dma_start(out=qkb_sb, in_=qkb_d.rearrange("(m p) -> p m", p=P))
        nc.sync.dma_start(out=pb_sb, in_=pb_d.rearrange("(m p) -> p m", p=P))
        nc.sync.dma_start(out=f1b_sb, in_=f1b_d.rearrange("(m p) -> p m", p=P))
        nc.sync.dma_start(out=f2b_sb, in_=f2b_d.rearrange("(m p) -> p m", p=P))

        # ================= Phase B: v projection + attention =================
        with tc.tile_pool(name="psB", space="PSUM", bufs=2) as psB:
            # ---- v = h @ vw (token-major), packed into vext with ones column ----
            nc.vector.memset(vext[:, :, :, HD:HD + 1], 1.0)
            for t in range(NT):
                pv = psB.tile([P, 2, 512], F32, tag="mm2", name=f"pv_{t}")
                for half, (c0, cw) in enumerate(((0, 512), (512, 256))):
                    for e in range(EC):
                        nc.tensor.matmul(
                            pv[:, half, :cw],
                            actT[:, e, t * P:(t + 1) * P],
                            vw_sb[:, e, c0:c0 + cw],
                            start=(e == 0),
                            stop=(e == EC - 1),
                        )
                nc.vector.tensor_copy(
                    out=vext[:, t, 0:8, 0:HD],
                    in_=pv[:, 0, :].rearrange("p (h d) -> p h d", d=HD),
                )
                nc.vector.tensor_copy(
                    out=vext[:, t, 8:12, 0:HD],
                    in_=pv[:, 1, 0:256].rearrange("p (h d) -> p h d", d=HD),
                )

            nc.sync.dma_start(out=pw_sb, in_=pw_d.rearrange("(kc p) e -> p kc e", p=P))

            # ---- per head-pair: qk projection (prefetched one pair ahead),
            # ---- then attention for 2 heads
            qkw_r = qkw_d.rearrange("(kc p) o -> p kc o", p=P)

            def emit_qk(hp):
                qkT = {}
                for role, m in (("q", hp), ("k", HP + hp)):
                    wch = work.tile([P, EC, P], BF16, tag="wchunk", name=f"qkw_{role}{hp}")
                    nc.sync.dma_start(out=wch, in_=qkw_r[:, :, m * P:(m + 1) * P])
                    dst = work.tile([P, SEQ], BF16, tag="qkT", bufs=4, name=f"{role}T_{hp}")
                    for n in range(NC2):
                        pqk = psB.tile([P, 512], F32, tag="pqk", bufs=2, name=f"pqk_{role}{hp}n{n}")
                        for e in range(EC):
                            nc.tensor.matmul(
                                pqk,
                                wch[:, e, :],
                                actT[:, e, n * 512:(n + 1) * 512],
                                start=(e == 0),
                                stop=(e == EC - 1),
                            )
                        nc.vector.tensor_scalar_add(
                            out=dst[:, n * 512:(n + 1) * 512],
                            in0=pqk,
                            scalar1=qkb_sb[:, m:m + 1],
                        )
                    qkT[role] = dst
                return qkT

            qkT = emit_qk(0)
            for hp in range(HP):
                cur = qkT
                if hp + 1 < HP:
                    qkT = emit_qk(hp + 1)

                ous = {}
                dpack = stat_pool.tile([4, 512], F32, tag="dpack", bufs=2, name=f"dp_{hp}")
                for sub in range(2):
                    h = 2 * hp + sub
                    doff = sub * HD
                    qs = cur["q"][doff:doff + HD, :]
                    ks = cur["k"][doff:doff + HD, :]
                    po = [
                        psB.tile([P, 512], F32, tag="oacc", bufs=2, name=f"po_h{h}n{n}")
                        for n in range(NC2)
                    ]
                    for mt in range(NT):
                        ps = psB.tile([P, 2, 512], F32, tag="mm2", name=f"ps_h{h}m{mt}")
                        for n in range(NC2):
                            nc.tensor.matmul(
                                ps[:, n, :],
                                ks[:, mt * P:(mt + 1) * P],
                                qs[:, n * 512:(n + 1) * 512],
                                start=True,
                                stop=True,
                            )
                        pp = work.tile([P, 2, 512], BF16, tag="ppair", bufs=4, name=f"pp_h{h}m{mt}")
                        nc.scalar.activation(out=pp, in_=ps, func=AF.Exp, scale=SCALE)
                        for n in range(NC2):
                            nc.tensor.matmul(
                                po[n][0:HD + 1, :],
                                vext[:, mt, h, :],
                                pp[:, n, :],
                                start=(mt == 0),
                                stop=(mt == NT - 1),
                            )
                    for n in range(NC2):
                        # copy out of PSUM right away so the accumulator bank
                        # recycles without waiting on the normalize chain
                        ou = work.tile([HD + 1, 512], F32, tag="ou", bufs=8, name=f"ou_h{h}n{n}")
                        nc.vector.tensor_copy(out=ou, in_=po[n][0:HD + 1, :])
                        idx = sub * NC2 + n
                        nc.sync.dma_start(out=dpack[idx:idx + 1, :], in_=ou[HD:HD + 1, :])
                        ous[idx] = ou
                # one exact reciprocal for the whole head-pair's denominators
                rpack = stat_pool.tile([4, 512], F32, tag="rpack", bufs=2, name=f"rp_{hp}")
                nc.vector.reciprocal(out=rpack, in_=dpack)
                for sub in range(2):
                    doff = sub * HD
                    for n in range(NC2):
                        idx = sub * NC2 + n
                        rtmp = stat_pool.tile([1, 512], F32, tag="rtmp", bufs=4, name=f"rt_{hp}i{idx}")
                        nc.sync.dma_start(out=rtmp, in_=rpack[idx:idx + 1, :])
                        rb = work.tile([HD, 512], F32, tag="rb", bufs=4, name=f"rb_{hp}i{idx}")
                        nc.gpsimd.partition_broadcast(rb, rtmp)
                        nc.vector.tensor_tensor(
                            out=oT[doff:doff + HD, hp, n * 512:(n + 1) * 512],
                            in0=ous[idx][0:HD, :],
                            in1=rb,
                            op=ALU.mult,
                        )

        # ====== Phase C: proj + residual + LN2, one 512-token chunk at a time ======
        with tc.tile_pool(name="psC", space="PSUM", bufs=2) as psC:
            for n in range(NC2):
                for me in range(EC):
                    ppr = psC.tile([P, 512], F32, tag="mm", name=f"ppr_{me}_{n}")
                    for kc in range(EC):
                        nc.tensor.matmul(
                            ppr,
                            pw_sb[:, kc, me * P:(me + 1) * P],
                            oT[:, kc, n * 512:(n + 1) * 512],
                            start=(kc == 0),
                            stop=(kc == EC - 1),
                        )
                    prn = work.tile([P, 512], BF16, tag="prn", name=f"prn_{me}_{n}")
                    nc.scalar.activation(
                        out=prn, in_=ppr, func=AF.Identity, bias=pb_sb[:, me:me + 1]
                    )
                    ptr = psC.tile([P, 4, P], BF16, tag="tr", name=f"trp_{me}_{n}")
                    for j in range(4):
                        nc.tensor.transpose(ptr[:, j, :], prn[:, j * P:(j + 1) * P], ident)
                    nc.vector.tensor_tensor(
                        out=x1[:, 4 * n:4 * n + 4, me * P:(me + 1) * P],
                        in0=x1[:, 4 * n:4 * n + 4, me * P:(me + 1) * P],
                        in1=ptr,
                        op=ALU.add,
                    )
                hs2, mvs2 = [], []
                for j in range(4):
                    t = 4 * n + j
                    mv = stat_pool.tile([P, 2], F32, tag="mv", bufs=NT, name=f"mv2_{t}")
                    stats = stat_pool.tile([P, 3, 6], F32, tag="stats", name=f"st2_{t}")
                    _ln_stats(nc, x1[:, t, :], mv, stats, eps_t)
                    mvs2.append(mv)
                for j in range(4):
                    t = 4 * n + j
                    h_t = work.tile([P, EMB], BF16, tag="h", bufs=NT, name=f"h2_{t}")
                    _ln_apply(nc, x1[:, t, :], h_t, mvs2[j])
                    hs2.append(h_t)
                for j in range(4):
                    _transpose_to_featmajor(nc, tc, psC, work, hs2[j], actT, 4 * n + j)
        h2T = actT

        # ================= Phase F: MLP + residual + output =================
        out_r = out_d.rearrange("(t p) e -> p t e", p=P)
        f1w_r = f1w_d.rearrange("(kc p) o -> p kc o", p=P)
        f2w_r = f2w_d.rearrange("(hc p) e -> p hc e", p=P)
        with tc.tile_pool(name="psF", space="PSUM", bufs=1) as psF:
            for n in range(NC2):
                acc = [
                    psF.tile([P, 2, 512], F32, tag=f"acc{i}", bufs=1, name=f"acc_{n}_{i}")
                    for i in range(3)
                ]

                def acc_sl(e):
                    return acc[e // 2][:, e % 2, :]

                for hc in range(HC):
                    w1 = work.tile([P, EC, P], BF16, tag="wchunk", name=f"f1w_{n}_{hc}")
                    nc.sync.dma_start(out=w1, in_=f1w_r[:, :, hc * P:(hc + 1) * P])
                    w2 = work.tile([P, EMB], BF16, tag="w2chunk", name=f"f2w_{n}_{hc}")
                    nc.sync.dma_start(out=w2, in_=f2w_r[:, hc, :])
                    pf1 = psF.tile([P, 512], F32, tag="f1", bufs=2, name=f"pf1_{n}_{hc}")
                    for e in range(EC):
                        nc.tensor.matmul(
                            pf1,
                            w1[:, e, :],
                            h2T[:, e, n * 512:(n + 1) * 512],
                            start=(e == 0),
                            stop=(e == EC - 1),
                        )
                    a = work.tile([P, 512], BF16, tag="act", name=f"act_{n}_{hc}")
                    nc.scalar.activation(
                        out=a, in_=pf1, func=AF.Gelu, bias=f1b_sb[:, hc:hc + 1]
                    )
                    for e in range(EC):
                        nc.tensor.matmul(
                            acc_sl(e),
                            w2[:, e * P:(e + 1) * P],
                            a,
                            start=(hc == 0),
                            stop=(hc == HC - 1),
                        )
                for e in range(EC):
                    fr = work.tile([P, 512], BF16, tag="prn", name=f"fr_{n}_{e}")
                    nc.scalar.activation(
                        out=fr, in_=acc_sl(e), func=AF.Identity, bias=f2b_sb[:, e:e + 1]
                    )
                    ptr = psF.tile([P, 4, P], BF16, tag="f1", bufs=2, name=f"trf_{n}_{e}")
                    for j in range(4):
                        nc.tensor.transpose(ptr[:, j, :], fr[:, j * P:(j + 1) * P], ident)
                    nc.vector.tensor_tensor(
                        out=x1[:, 4 * n:4 * n + 4, e * P:(e + 1) * P],
                        in0=x1[:, 4 * n:4 * n + 4, e * P:(e + 1) * P],
                        in1=ptr,
                        op=ALU.add,
                    )
                for j in range(4):
                    t = 4 * n + j
                    nc.sync.dma_start(out=out_r[:, t, :], in_=x1[:, t, :])


def fold_inputs(inputs):
    """Fold LN gamma/beta and v-bias into downstream weights (exact math).

    Returns the dict of effective tensors the kernel consumes.
    """
    f = {k: np.asarray(v, dtype=np.float32) for k, v in inputs.items()}
    qkw = f["ln1_g"][:, None] * f["qk_w"]
    qkb = f["ln1_b"] @ f["qk_w"]
    vw = f["ln1_g"][:, None] * f["v_w"]
    vb = f["ln1_b"] @ f["v_w"]
    # softmax rows sum to 1 => o = attn @ (v + 1 vb^T) = attn@v + vb
    pb = f["proj_b"] + vb @ f["proj_w"]
    f1w = f["ln2_g"][:, None] * f["fc1_w"]
    f1b = f["fc1_b"] + f["ln2_b"] @ f["fc1_w"]
    import ml_dtypes

    bf16 = ml_dtypes.bfloat16
    return {
        "qkw": np.ascontiguousarray(qkw.astype(bf16)),
        "qkb": np.ascontiguousarray(qkb),
        "vw": np.ascontiguousarray(vw.astype(bf16)),
        "pw": np.ascontiguousarray(f["proj_w"].astype(bf16)),
        "pb": np.ascontiguousarray(pb),
        "f1w": np.ascontiguousarray(f1w.astype(bf16)),
        "f1b": np.ascontiguousarray(f1b),
        "f2w": np.ascontiguousarray(f["fc2_w"].astype(bf16)),
        "f2b": np.ascontiguousarray(f["fc2_b"]),
    }


_INPUT_SHAPES = {
    "x": (SEQ, EMB),
    "qkw": (EMB, 2 * EMB),
    "qkb": (2 * EMB,),
    "vw": (EMB, EMB),
    "pw": (EMB, EMB),
    "pb": (EMB,),
    "f1w": (EMB, MLPD),
    "f1b": (MLPD,),
    "f2w": (MLPD, EMB),
    "f2b": (EMB,),
}

_N_CORES = 8
_compiled = {}


def _build_nc(num_devices=_N_CORES):
    import concourse.tile as tile
    from concourse import bacc

    nc = bacc.Bacc(
        "TRN2", target_bir_lowering=False, debug=False, num_devices=num_devices
    )
    _BF16_INPUTS = {"qkw", "vw", "pw", "f1w", "f2w"}
    ins = {
        name: nc.dram_tensor(
            name, list(shape), BF16 if name in _BF16_INPUTS else F32,
            kind="ExternalInput",
        ).ap()
        for name, shape in _INPUT_SHAPES.items()
    }
    out = nc.dram_tensor("out", [SEQ, EMB], F32, kind="ExternalOutput").ap()
    with tile.TileContext(nc) as tc:
        build_block(tc, {"out": out}, ins)
    nc.compile()
    return nc


def kernel(**inputs):
    """Full-input entry point: x [8, 1024, 768] + weights -> [8, 1024, 768]."""
    from concourse.bass_utils import run_bass_kernel_spmd

    if "nc" not in _compiled:
        _compiled["nc"] = _build_nc()
    nc = _compiled["nc"]

    x = np.asarray(inputs["x"], dtype=np.float32)
    folded = fold_inputs({k: v for k, v in inputs.items() if k != "x"})
    in_maps = [
        {"x": np.ascontiguousarray(x[c]), **folded} for c in range(_N_CORES)
    ]
    res = run_bass_kernel_spmd(nc, in_maps, core_ids=list(range(_N_CORES)))
    return np.stack([res.results[c]["out"] for c in range(_N_CORES)]).astype(
        np.float32
    )



# revision 5
# speedup vs baseline: 1.1471x; 1.1471x over previous
"""Transformer block kernel for TRN2 (Bass/Tile), one batch element per core.

v2: fp8 (e4m3) DoubleRow matmuls for all linear layers + PV, row-tiled
(64x128) score matmuls, all weights SBUF-resident in fp8.

Computes (per core, x [1024, 768] bf16 in, f32 out):
    h  = LN(x) (gamma/beta pre-folded into weights on host)      -> fp8
    qk = h @ (64*qkw fp8); q,k bf16 = psum/64 + qkb              (DoubleRow)
    v  = h @ (64*vw fp8) / 64 -> fp8 in vext (with ones col)     (DoubleRow)
    S^T[m,n] = (k_m . q_n)/8 - 1 ; P = exp(S^T) -> fp8           (64x128 row-tiled pairs)
    oe = [v;1]^T @ P  (fp8 DoubleRow over mt pairs); o^T = oe[0:64]/oe[64]
    x1 = x + o @ (64*pw fp8)/64 + pb
    h2 = LN2(x1) -> fp8
    out = x1 + gelu(h2 @ 64*f1w / 64 + f1b) @ 64*f2w / 64 + f2b  (DoubleRow)

Layout: "feature-major" tensors are [feat_on_partitions, tokens]; token-major
are [tokens_on_partitions, feat]. LN / residual token-major; matmuls contract
over partitions so projections run feature-major. Weights pre-scaled by 64 on
host so fp8 quantization of small weights stays in the normal range; the /64
is folded into the PSUM-evacuation activation's scale.
"""

import sys
from contextlib import ExitStack

if "/opt/trn_rl_repo" not in sys.path:
    sys.path.insert(0, "/opt/trn_rl_repo")

import numpy as np

import concourse.bass as bass
import concourse.mybir as mybir
from concourse.masks import make_identity

F32 = mybir.dt.float32
BF16 = mybir.dt.bfloat16
F8 = mybir.dt.float8e4
AF = mybir.ActivationFunctionType
ALU = mybir.AluOpType
DR = mybir.MatmulPerfMode.DoubleRow

P = 128
EMB = 768
SEQ = 1024
NH = 12
HD = 64
MLPD = 3072
EC = EMB // P      # 6 embedding chunks
NT = SEQ // P      # 8 token tiles
NC2 = SEQ // 512   # 2 token n-chunks
HC = MLPD // P     # 24 hidden chunks
HP = NH // 2       # 6 head pairs
EPS = 1e-5
SCALE = HD ** -0.5
WS = 64.0          # host-side weight scale for fp8
IWS = 1.0 / WS
VP = 80            # padded per-head stride in vext (64 v + 1 ones + 15 pad)


def _ln_stats(nc, x_ap, mv_out, stats):
    """bn stats for one [128, EMB] tile; mv_out = [mean, var] (var raw)."""
    xg = x_ap.rearrange("p (g d) -> p g d", d=256)
    for g in range(3):
        nc.vector.bn_stats(out=stats[:, g, :], in_=xg[:, g, :])
    nc.vector.bn_aggr(out=mv_out, in_=stats)


def _transpose_to_featmajor(nc, pool_ps, src_tok, dstT, t, ident):
    """PE-transpose token-major src_tok [128, EMB] bf16 into fp8 dstT."""
    for group_start, group_n in ((0, 4), (4, 2)):
        ptr = pool_ps.tile([P, 4 * P], BF16, tag="tr", bufs=2,
                           name=f"ptr_t{t}_{group_start}")
        for j in range(group_n):
            e = group_start + j
            nc.tensor.transpose(
                ptr[:, j * P:(j + 1) * P],
                src_tok[:, e * P:(e + 1) * P],
                ident,
            )
        nc.scalar.copy(
            out=dstT[:, group_start:group_start + group_n, t * P:(t + 1) * P],
            in_=ptr[:, :group_n * P].rearrange("p (j q) -> p j q", q=P),
        )


def build_block(tc, outs, ins):
    nc = tc.nc
    x_d = ins["x"]
    qkw_d, qkb_d = ins["qkw"], ins["qkb"]
    vw_d = ins["vw"]
    pw_d, pb_d = ins["pw"], ins["pb"]
    f1w_d, f1b_d = ins["f1w"], ins["f1b"]
    f2w_d, f2b_d = ins["f2w"], ins["f2b"]
    out_d = ins_out = outs["out"]

    with ExitStack() as ctx:
        consts = ctx.enter_context(tc.tile_pool(name="consts", bufs=1))
        ident = consts.tile([P, P], BF16)
        make_identity(nc, ident)
        eps_t = consts.tile([P, 1], F32)
        nc.vector.memset(eps_t, EPS)
        negone = consts.tile([P, 1], F32)
        nc.vector.memset(negone, -1.0)
        qkb_sb = consts.tile([P, 2 * EC], F32)
        pb_sb = consts.tile([P, EC], F32)
        f1b_sb = consts.tile([P, HC], F32)
        f2b_sb = consts.tile([P, EC], F32)

        # Persistent SBUF tensors
        glob = ctx.enter_context(tc.tile_pool(name="glob", bufs=1))
        x1 = glob.tile([P, NT, EMB], BF16)           # residual stream
        actT = glob.tile([P, EC, SEQ], F8)           # hT, later h2T
        oT = glob.tile([P, EC, SEQ], F8)             # attention out, feature-major
        vext = glob.tile([P, NT, NH, VP], F8)        # v tokens-major + ones col
        qkT = glob.tile([P, 2 * EC, SEQ], BF16)      # q (0..5) / k (6..11) feature-major

        wpool = ctx.enter_context(tc.tile_pool(name="wpool", bufs=1))
        qkw_sb = wpool.tile([P, EC, 2 * EMB], F8)
        vw_sb = wpool.tile([P, EC, EMB], F8)
        pw_sb = wpool.tile([P, EC, EMB], F8)
        f1w_sb = wpool.tile([P, EC, MLPD], F8)
        f2w_sb = wpool.tile([P, HC, EMB], F8)

        work = ctx.enter_context(tc.tile_pool(name="work", bufs=3))
        stat_pool = ctx.enter_context(tc.tile_pool(name="stat", bufs=4))

        # ---- load x into x1 ----
        x_r = x_d.rearrange("(t p) e -> p t e", p=P)
        for t in range(NT):
            nc.sync.dma_start(out=x1[:, t, :], in_=x_r[:, t, :])

        # ================= Phase A: LN1 + transpose to hT (fp8) =================
        with tc.tile_pool(name="psA", space="PSUM", bufs=2) as psA:
            mv1 = stat_pool.tile([P, NT, 2], F32, tag="mv1", bufs=1)
            for t in range(NT):
                stats = stat_pool.tile([P, 3, 6], F32, tag="stats", name=f"st1_{t}")
                _ln_stats(nc, x1[:, t, :], mv1[:, t, :], stats)
            # rstd = 1/sqrt(var+eps), batched over the 8 tiles
            nc.scalar.activation(
                out=mv1[:, :, 1], in_=mv1[:, :, 1], func=AF.Sqrt, bias=eps_t, scale=1.0
            )
            nc.vector.reciprocal(out=mv1[:, :, 1], in_=mv1[:, :, 1])
            for t in range(NT):
                h_t = work.tile([P, EMB], BF16, tag="h", bufs=4, name=f"h_{t}")
                nc.vector.tensor_scalar(
                    out=h_t, in0=x1[:, t, :],
                    scalar1=mv1[:, t, 0:1], scalar2=mv1[:, t, 1:2],
                    op0=ALU.subtract, op1=ALU.mult,
                )
                _transpose_to_featmajor(nc, psA, h_t, actT, t, ident)

        # weights / biases (emitted after x+LN so the x DMAs win the queues)
        nc.sync.dma_start(out=qkw_sb, in_=qkw_d.rearrange("(kc p) o -> p kc o", p=P))
        nc.sync.dma_start(out=vw_sb, in_=vw_d.rearrange("(kc p) o -> p kc o", p=P))
        nc.sync.dma_start(out=qkb_sb, in_=qkb_d.rearrange("(m p) -> p m", p=P))
        nc.sync.dma_start(out=pb_sb, in_=pb_d.rearrange("(m p) -> p m", p=P))
        nc.sync.dma_start(out=f1b_sb, in_=f1b_d.rearrange("(m p) -> p m", p=P))
        nc.sync.dma_start(out=f2b_sb, in_=f2b_d.rearrange("(m p) -> p m", p=P))
        nc.scalar.dma_start(out=pw_sb, in_=pw_d.rearrange("(kc p) o -> p kc o", p=P))
        nc.scalar.dma_start(out=f1w_sb, in_=f1w_d.rearrange("(kc p) o -> p kc o", p=P))
        nc.scalar.dma_start(out=f2w_sb, in_=f2w_d.rearrange("(kc p) o -> p kc o", p=P))

        # ============ Phase B0: v projection + qk projections (DoubleRow) ============
        with tc.tile_pool(name="psV", space="PSUM", bufs=2) as psV:
            nc.vector.memset(vext[:, :, :, HD:HD + 1], 1.0)
            for t in range(NT):
                pv = psV.tile([P, 2, 512], F32, tag="pv", name=f"pv_{t}")
                for half, (c0, cw) in enumerate(((0, 512), (512, 256))):
                    for ep in range(3):
                        nc.tensor.matmul(
                            pv[:, half, :cw],
                            actT[:, 2 * ep:2 * ep + 2, t * P:(t + 1) * P],
                            vw_sb[:, 2 * ep:2 * ep + 2, c0:c0 + cw],
                            start=(ep == 0), stop=(ep == 2),
                            perf_mode=DR,
                        )
                nc.vector.tensor_scalar(
                    out=vext[:, t, 0:8, 0:HD],
                    in0=pv[:, 0, :].rearrange("p (h d) -> p h d", d=HD),
                    scalar1=IWS, scalar2=None, op0=ALU.mult,
                )
                nc.vector.tensor_scalar(
                    out=vext[:, t, 8:12, 0:HD],
                    in0=pv[:, 1, 0:256].rearrange("p (h d) -> p h d", d=HD),
                    scalar1=IWS, scalar2=None, op0=ALU.mult,
                )
            for j in range(2 * EC):
                for n in range(NC2):
                    pqk = psV.tile([P, 512], F32, tag="pqk", bufs=2, name=f"pqk_{j}_{n}")
                    for ep in range(3):
                        nc.tensor.matmul(
                            pqk,
                            qkw_sb[:, 2 * ep:2 * ep + 2, j * P:(j + 1) * P],
                            actT[:, 2 * ep:2 * ep + 2, n * 512:(n + 1) * 512],
                            start=(ep == 0), stop=(ep == 2),
                            perf_mode=DR,
                        )
                    nc.scalar.activation(
                        out=qkT[:, j, n * 512:(n + 1) * 512], in_=pqk,
                        func=AF.Identity, bias=qkb_sb[:, j:j + 1], scale=IWS,
                    )

        # ================= Phase B1: attention, one head-pair at a time =================
        # S: 64x128 row-tiled bf16 matmul pairs (head0 on partitions 0-63 / tile
        # (0,0), head1 on 64-127 / tile (64,0)), emitted back-to-back so they run
        # concurrently in the PE array. PV: fp8 DoubleRow over mt-tile pairs with
        # the ones column producing the softmax denominator in row 64.
        with tc.tile_pool(name="psS", space="PSUM", bufs=4) as psS, \
             tc.tile_pool(name="psPO", space="PSUM", bufs=4) as psPO:
            for hp in range(HP):
                pps = {}
                for sub in range(2):
                    for mp in range(4):
                        pps[(sub, mp)] = work.tile(
                            [P, 2, NC2, 512], F8, tag="pp", bufs=10,
                            name=f"pp_h{hp}s{sub}_m{mp}",
                        )
                for mt in range(NT):
                    sS = {}
                    for n in range(NC2):
                        for sub in range(2):
                            base = sub * HD
                            ps = psS.tile([P, 512], F32, tag="s",
                                          name=f"s_{hp}_{sub}_{mt}_{n}")
                            nc.tensor.matmul(
                                ps,
                                qkT[base:base + HD, EC + hp, mt * P:(mt + 1) * P],
                                qkT[base:base + HD, hp, n * 512:(n + 1) * 512],
                                start=True, stop=True,
                            )
                            sS[(sub, n)] = ps
                    for sub in range(2):
                        for n in range(NC2):
                            nc.scalar.activation(
                                out=pps[(sub, mt // 2)][:, mt % 2, n, :],
                                in_=sS[(sub, n)], func=AF.Exp,
                                scale=SCALE, bias=negone,
                            )
                ous = {}
                dpack = stat_pool.tile([4, 512], F32, tag="dpack", bufs=2,
                                       name=f"dp_{hp}")
                for sub in range(2):
                    h = 2 * hp + sub
                    for n in range(NC2):
                        po = psPO.tile([P, 512], F32, tag="po",
                                       name=f"po_{hp}_{sub}_{n}")
                        for mp in range(4):
                            nc.tensor.matmul(
                                po[0:HD + 1, :],
                                vext[:, 2 * mp:2 * mp + 2, h, 0:HD + 1],
                                pps[(sub, mp)][:, :, n, :],
                                start=(mp == 0), stop=(mp == 3),
                                perf_mode=DR,
                            )
                        idx = sub * NC2 + n
                        ou = work.tile([HD + 1, 512], F32, tag="ou", bufs=8,
                                       name=f"ou_{hp}_{idx}")
                        nc.vector.tensor_copy(out=ou, in_=po[0:HD + 1, :])
                        nc.sync.dma_start(out=dpack[idx:idx + 1, :], in_=ou[HD:HD + 1, :])
                        ous[idx] = ou
                rpack = stat_pool.tile([4, 512], F32, tag="rpack", bufs=2,
                                       name=f"rp_{hp}")
                nc.vector.reciprocal(out=rpack, in_=dpack)
                for sub in range(2):
                    doff = sub * HD
                    for n in range(NC2):
                        idx = sub * NC2 + n
                        rtmp = stat_pool.tile([1, 512], F32, tag="rtmp", bufs=4,
                                              name=f"rt_{hp}_{idx}")
                        nc.sync.dma_start(out=rtmp, in_=rpack[idx:idx + 1, :])
                        rb = work.tile([HD, 512], F32, tag="rb", bufs=4,
                                       name=f"rb_{hp}_{idx}")
                        nc.gpsimd.partition_broadcast(rb, rtmp)
                        nc.vector.tensor_tensor(
                            out=oT[doff:doff + HD, hp, n * 512:(n + 1) * 512],
                            in0=ous[idx][0:HD, :], in1=rb, op=ALU.mult,
                        )

        # ====== Phase C: proj + residual + LN2, one 512-token chunk at a time ======
        with tc.tile_pool(name="psC", space="PSUM", bufs=1) as psC:
            for n in range(NC2):
                prns = []
                for me in range(EC):
                    ppr = psC.tile([P, 512], F32, tag="ppr", bufs=6,
                                   name=f"ppr_{me}_{n}")
                    for kp in range(3):
                        nc.tensor.matmul(
                            ppr,
                            pw_sb[:, 2 * kp:2 * kp + 2, me * P:(me + 1) * P],
                            oT[:, 2 * kp:2 * kp + 2, n * 512:(n + 1) * 512],
                            start=(kp == 0), stop=(kp == 2),
                            perf_mode=DR,
                        )
                    prn = work.tile([P, 512], BF16, tag="prn", name=f"prn_{me}_{n}")
                    nc.scalar.activation(
                        out=prn, in_=ppr, func=AF.Identity,
                        bias=pb_sb[:, me:me + 1], scale=IWS,
                    )
                    prns.append(prn)
                for me in range(EC):
                    ptr = psC.tile([P, 4, P], BF16, tag="tr", bufs=2,
                                   name=f"trp_{me}_{n}")
                    for j in range(4):
                        nc.tensor.transpose(ptr[:, j, :], prns[me][:, j * P:(j + 1) * P], ident)
                    nc.vector.tensor_tensor(
                        out=x1[:, 4 * n:4 * n + 4, me * P:(me + 1) * P],
                        in0=x1[:, 4 * n:4 * n + 4, me * P:(me + 1) * P],
                        in1=ptr, op=ALU.add,
                    )
                mv2 = stat_pool.tile([P, 4, 2], F32, tag="mv2", bufs=2, name=f"mv2_{n}")
                for j in range(4):
                    stats = stat_pool.tile([P, 3, 6], F32, tag="stats", name=f"st2_{4*n+j}")
                    _ln_stats(nc, x1[:, 4 * n + j, :], mv2[:, j, :], stats)
                nc.scalar.activation(
                    out=mv2[:, :, 1], in_=mv2[:, :, 1], func=AF.Sqrt, bias=eps_t, scale=1.0
                )
                nc.vector.reciprocal(out=mv2[:, :, 1], in_=mv2[:, :, 1])
                for j in range(4):
                    t = 4 * n + j
                    h_t = work.tile([P, EMB], BF16, tag="h", bufs=4, name=f"h2_{t}")
                    nc.vector.tensor_scalar(
                        out=h_t, in0=x1[:, t, :],
                        scalar1=mv2[:, j, 0:1], scalar2=mv2[:, j, 1:2],
                        op0=ALU.subtract, op1=ALU.mult,
                    )
                    _transpose_to_featmajor(nc, psC, h_t, actT, t, ident)
        h2T = actT

        # ================= Phase F: MLP + residual + output =================
        out_r = out_d.rearrange("(t p) e -> p t e", p=P)
        with tc.tile_pool(name="psF", space="PSUM", bufs=1) as psF:
            for n in range(NC2):
                acc = [
                    psF.tile([P, 2, 512], F32, tag=f"acc{i}", bufs=1, name=f"acc_{n}_{i}")
                    for i in range(3)
                ]

                def acc_sl(e):
                    return acc[e // 2][:, e % 2, :]

                for hcp in range(HC // 2):
                    apair = work.tile([P, 2, 512], F8, tag="apair", bufs=3,
                                      name=f"a_{n}_{hcp}")
                    for u in range(2):
                        hc = 2 * hcp + u
                        pf1 = psF.tile([P, 512], F32, tag="f1", bufs=2,
                                       name=f"pf1_{n}_{hc}")
                        for ep in range(3):
                            nc.tensor.matmul(
                                pf1,
                                f1w_sb[:, 2 * ep:2 * ep + 2, hc * P:(hc + 1) * P],
                                h2T[:, 2 * ep:2 * ep + 2, n * 512:(n + 1) * 512],
                                start=(ep == 0), stop=(ep == 2),
                                perf_mode=DR,
                            )
                        nc.scalar.activation(
                            out=apair[:, u, :], in_=pf1, func=AF.Gelu,
                            bias=f1b_sb[:, hc:hc + 1], scale=IWS,
                        )
                    for e in range(EC):
                        nc.tensor.matmul(
                            acc_sl(e),
                            f2w_sb[:, 2 * hcp:2 * hcp + 2, e * P:(e + 1) * P],
                            apair,
                            start=(hcp == 0), stop=(hcp == HC // 2 - 1),
                            perf_mode=DR,
                        )
                fout = work.tile([P, 4, EMB], F32, tag="fout", bufs=2, name=f"fo_{n}")
                for e in range(EC):
                    fr = work.tile([P, 512], BF16, tag="prn", name=f"fr_{n}_{e}")
                    nc.scalar.activation(
                        out=fr, in_=acc_sl(e), func=AF.Identity,
                        bias=f2b_sb[:, e:e + 1], scale=IWS,
                    )
                    ptr = psF.tile([P, 4, P], BF16, tag="f1", bufs=2, name=f"trf_{n}_{e}")
                    for j in range(4):
                        nc.tensor.transpose(ptr[:, j, :], fr[:, j * P:(j + 1) * P], ident)
                    nc.vector.tensor_tensor(
                        out=fout[:, :, e * P:(e + 1) * P],
                        in0=x1[:, 4 * n:4 * n + 4, e * P:(e + 1) * P],
                        in1=ptr, op=ALU.add,
                    )
                for j in range(4):
                    t = 4 * n + j
                    nc.sync.dma_start(out=out_r[:, t, :], in_=fout[:, j, :])


def fold_inputs(inputs):
    """Fold LN gamma/beta and v-bias into downstream weights; scale weights by
    WS=64 and quantize to fp8 e4m3 (IEEE, max 240 — matches TRN FP8_EXP4).

    Returns the dict of effective tensors the kernel consumes.
    """
    import ml_dtypes

    f = {k: np.asarray(v, dtype=np.float32) for k, v in inputs.items()}
    qkw = f["ln1_g"][:, None] * f["qk_w"]
    qkb = f["ln1_b"] @ f["qk_w"]
    vw = f["ln1_g"][:, None] * f["v_w"]
    vb = f["ln1_b"] @ f["v_w"]
    # softmax rows sum to 1 => o = attn @ (v + 1 vb^T) = attn@v + vb
    pb = f["proj_b"] + vb @ f["proj_w"]
    f1w = f["ln2_g"][:, None] * f["fc1_w"]
    f1b = f["fc1_b"] + f["ln2_b"] @ f["fc1_w"]

    fp8 = ml_dtypes.float8_e4m3

    def w8(a):
        return np.ascontiguousarray(np.clip(a * WS, -240.0, 240.0).astype(fp8))

    return {
        "qkw": w8(qkw),
        "qkb": np.ascontiguousarray(qkb),
        "vw": w8(vw),
        "pw": w8(f["proj_w"]),
        "pb": np.ascontiguousarray(pb),
        "f1w": w8(f1w),
        "f1b": np.ascontiguousarray(f1b),
        "f2w": w8(f["fc2_w"]),
        "f2b": np.ascontiguousarray(f["fc2_b"]),
    }


def prep_core_inputs(inputs):
    """Full-input dict -> list of per-core in_maps (x cast to bf16)."""
    import ml_dtypes

    x = np.asarray(inputs["x"], dtype=np.float32).astype(ml_dtypes.bfloat16)
    folded = fold_inputs({k: v for k, v in inputs.items() if k != "x"})
    return [
        {"x": np.ascontiguousarray(x[c]), **folded} for c in range(_N_CORES)
    ]


_INPUT_SHAPES = {
    "x": (SEQ, EMB),
    "qkw": (EMB, 2 * EMB),
    "qkb": (2 * EMB,),
    "vw": (EMB, EMB),
    "pw": (EMB, EMB),
    "pb": (EMB,),
    "f1w": (EMB, MLPD),
    "f1b": (MLPD,),
    "f2w": (MLPD, EMB),
    "f2b": (EMB,),
}

_F8_INPUTS = {"qkw", "vw", "pw", "f1w", "f2w"}
_BF16_INPUTS = {"x"}

_N_CORES = 8
_compiled = {}


def _build_nc(num_devices=_N_CORES):
    import concourse.tile as tile
    from concourse import bacc

    nc = bacc.Bacc(
        "TRN2", target_bir_lowering=False, debug=False, num_devices=num_devices
    )

    def dt_of(name):
        if name in _F8_INPUTS:
            return F8
        if name in _BF16_INPUTS:
            return BF16
        return F32

    ins = {
        name: nc.dram_tensor(
            name, list(shape), dt_of(name), kind="ExternalInput"
        ).ap()
        for name, shape in _INPUT_SHAPES.items()
    }
    out = nc.dram_tensor("out", [SEQ, EMB], F32, kind="ExternalOutput").ap()
    with tile.TileContext(nc) as tc:
        build_block(tc, {"out": out}, ins)
    nc.compile()
    return nc


def kernel(**inputs):
    """Full-input entry point: x [8, 1024, 768] + weights -> [8, 1024, 768]."""
    from concourse.bass_utils import run_bass_kernel_spmd

    if "nc" not in _compiled:
        _compiled["nc"] = _build_nc()
    nc = _compiled["nc"]

    in_maps = prep_core_inputs(inputs)
    res = run_bass_kernel_spmd(nc, in_maps, core_ids=list(range(_N_CORES)))
    return np.stack([res.results[c]["out"] for c in range(_N_CORES)]).astype(
        np.float32
    )


# revision 17
# speedup vs baseline: 1.1516x; 1.0039x over previous
"""Transformer block kernel for TRN2 (Bass/Tile), one batch element per core.

v3: fp8 (e4m3) DoubleRow matmuls for all linear layers + PV, row-tiled
(64x128) score matmul pairs, all weights SBUF-resident in fp8, softmax exp
split between ScalarE (exact exp -> fp8) and VectorE (Schraudolph-style
direct fp8-bit synthesis: bits = round(A*s + B) written as int8, read as fp8;
softmax renormalization cancels most of the approximation error), PE kept
warm with junk matmuls during the DMA/LN prologue.

Computes (per core, x [1024, 768] bf16 in, f32 out):
    h  = LN(x) (gamma/beta pre-folded into weights on host)      -> fp8
    qk = h @ (64*qkw fp8); q,k bf16 = psum/64 + qkb              (DoubleRow)
    v  = h @ (64*vw fp8) / 64 -> fp8 in vext (with ones col)     (DoubleRow)
    S^T[m,n] = (k_m . q_n)/8 - 1 ; P = exp(S^T) -> fp8           (64x128 row-tiled)
    oe = [v;1]^T @ P  (fp8 DoubleRow over mt pairs); o^T = oe[0:64]/oe[64]
    x1 = x + o @ (64*pw fp8)/64 + pb
    h2 = LN2(x1) -> fp8
    out = x1 + gelu(h2 @ 64*f1w / 64 + f1b) @ 64*f2w / 64 + f2b  (DoubleRow)
"""

import sys
from contextlib import ExitStack

if "/opt/trn_rl_repo" not in sys.path:
    sys.path.insert(0, "/opt/trn_rl_repo")

import math

import numpy as np

import concourse.bass as bass
import concourse.mybir as mybir
from concourse.masks import make_identity

F32 = mybir.dt.float32
BF16 = mybir.dt.bfloat16
F8 = mybir.dt.float8e4
I8 = mybir.dt.int8
AF = mybir.ActivationFunctionType
ALU = mybir.AluOpType
DR = mybir.MatmulPerfMode.DoubleRow

P = 128
EMB = 768
SEQ = 1024
NH = 12
HD = 64
MLPD = 3072
EC = EMB // P      # 6 embedding chunks
NT = SEQ // P      # 8 token tiles
NC2 = SEQ // 512   # 2 token n-chunks
HC = MLPD // P     # 24 hidden chunks
HP = NH // 2       # 6 head pairs
EPS = 1e-5
SCALE = HD ** -0.5
WS = 64.0          # host-side weight scale for fp8
IWS = 1.0 / WS
VP = 80            # padded per-head stride in vext (64 v + 1 ones + 15 pad)

# fp8-bit exp: for score s (pre-scale), pp_bits = round(EA*s + EB) gives
# fp8e4 bits approximating exp(s*SCALE - 1).  EA = 8*SCALE/ln2,
# EB = 56 - 0.5 - 8/ln2 (the -0.5 centers round-to-nearest error).
EA = 8.0 * SCALE / math.log(2.0)
EB = 56.0 - 0.5 - 8.0 / math.log(2.0)


def _ln_stats(nc, x_ap, mv_out, stats):
    """bn stats for one [128, EMB] tile; mv_out = [P,2,1] slice ([mean, var])."""
    xg = x_ap.rearrange("p (g d) -> p g d", d=256)
    for g in range(3):
        nc.vector.bn_stats(out=stats[:, g, :], in_=xg[:, g, :])
    nc.vector.bn_aggr(out=mv_out, in_=stats)


def _transpose_to_featmajor(nc, pool_ps, src_tok, dstT, t, ident):
    """PE-transpose token-major src_tok [128, EMB] bf16 into fp8 dstT."""
    for group_start, group_n in ((0, 4), (4, 2)):
        ptr = pool_ps.tile([P, 4 * P], BF16, tag="tr", bufs=2,
                           name=f"ptr_t{t}_{group_start}")
        for j in range(group_n):
            e = group_start + j
            nc.tensor.transpose(
                ptr[:, j * P:(j + 1) * P],
                src_tok[:, e * P:(e + 1) * P],
                ident,
            )
        nc.scalar.copy(
            out=dstT[:, group_start:group_start + group_n, t * P:(t + 1) * P],
            in_=ptr[:, :group_n * P].rearrange("p (j q) -> p j q", q=P),
        )


def build_block(tc, outs, ins):
    nc = tc.nc
    x_d = ins["x"]
    qkw_d, qkb_d = ins["qkw"], ins["qkb"]
    vw_d = ins["vw"]
    pw_d, pb_d = ins["pw"], ins["pb"]
    f1w_d, f1b_d = ins["f1w"], ins["f1b"]
    f2w_d, f2b_d = ins["f2w"], ins["f2b"]
    out_d = outs["out"]

    with ExitStack() as ctx:
        consts = ctx.enter_context(tc.tile_pool(name="consts", bufs=1))
        ident = consts.tile([P, P], BF16)
        make_identity(nc, ident)
        eps_t = consts.tile([P, 1], F32)
        nc.vector.memset(eps_t, EPS)
        negone = consts.tile([P, 1], F32)
        nc.vector.memset(negone, -1.0)
        qkb_sb = consts.tile([P, 2 * EC], F32)
        pb_sb = consts.tile([P, EC], F32)
        f1b_sb = consts.tile([P, HC], F32)
        f2b_sb = consts.tile([P, EC], F32)
        wjunk = consts.tile([P, 2, P], F8)
        ajunk = consts.tile([P, 2, 512], F8)
        nc.vector.memset(wjunk, 0.0)
        nc.vector.memset(ajunk, 0.0)

        # Persistent SBUF tensors
        glob = ctx.enter_context(tc.tile_pool(name="glob", bufs=1))
        x1 = glob.tile([P, NT, EMB], BF16)           # residual stream
        actT = glob.tile([P, EC, SEQ], F8)           # hT, later h2T
        oT = glob.tile([P, EC, SEQ], F8)             # attention out, feature-major
        vext = glob.tile([P, NT, NH, VP], F8)        # v tokens-major + ones col
        qkT = glob.tile([P, 2 * EC, SEQ], F8)        # q (0..5) / k (6..11) feature-major

        wpool = ctx.enter_context(tc.tile_pool(name="wpool", bufs=1))
        qkw_sb = wpool.tile([P, EC, 2 * EMB], F8)
        vw_sb = wpool.tile([P, EC, EMB], F8)
        pw_sb = wpool.tile([P, EC, EMB], F8)
        f1w_sb = wpool.tile([P, EC, MLPD], F8)
        f2w_sb = wpool.tile([P, HC, EMB], F8)

        work = ctx.enter_context(tc.tile_pool(name="work", bufs=3))
        stat_pool = ctx.enter_context(tc.tile_pool(name="stat", bufs=4))

        # ---- load x into x1 ----
        x_r = x_d.rearrange("(t p) e -> p t e", p=P)
        for t in range(NT):
            nc.sync.dma_start(out=x1[:, t, :], in_=x_r[:, t, :])

        # ================= Phase A: LN1 + transpose to hT (fp8) =================
        with tc.tile_pool(name="psA", space="PSUM", bufs=2) as psA:
            # PE warm-up: HAM needs ~3.4us of matmul activity to unthrottle the
            # clock; transposes don't count. Burn junk fp8 matmuls while the x
            # DMA + LN stats run so phase A's transposes execute at 2.4 GHz.
            pwarm = psA.tile([P, 512], F32, tag="warm", bufs=1, name="pwarm")
            for w in range(12):
                nc.tensor.matmul(pwarm, wjunk, ajunk, start=True, stop=True,
                                 perf_mode=DR)

            mv1 = stat_pool.tile([P, 2, NT], F32, tag="mv1", bufs=1)
            for t in range(NT):
                stats = stat_pool.tile([P, 3, 6], F32, tag="stats", name=f"st1_{t}")
                _ln_stats(nc, x1[:, t, :], mv1[:, :, t], stats)
            # rstd = 1/sqrt(var+eps), batched over the 8 tiles (contiguous row)
            nc.scalar.activation(
                out=mv1[:, 1, :], in_=mv1[:, 1, :], func=AF.Sqrt, bias=eps_t, scale=1.0
            )
            nc.vector.reciprocal(out=mv1[:, 1, :], in_=mv1[:, 1, :])
            for t in range(NT):
                h_t = work.tile([P, EMB], BF16, tag="h", bufs=4, name=f"h_{t}")
                nc.vector.tensor_scalar(
                    out=h_t, in0=x1[:, t, :],
                    scalar1=mv1[:, 0:1, t], scalar2=mv1[:, 1:2, t],
                    op0=ALU.subtract, op1=ALU.mult,
                )
                _transpose_to_featmajor(nc, psA, h_t, actT, t, ident)
                # keep HAM warm through the transpose-only stretch
                nc.tensor.matmul(pwarm, wjunk, ajunk, start=True, stop=True,
                                 perf_mode=DR)

        # weights / biases (emitted after x+LN so the x DMAs win the queues)
        nc.sync.dma_start(out=qkw_sb, in_=qkw_d.rearrange("(kc p) o -> p kc o", p=P))
        nc.sync.dma_start(out=vw_sb, in_=vw_d.rearrange("(kc p) o -> p kc o", p=P))
        nc.sync.dma_start(out=qkb_sb, in_=qkb_d.rearrange("(m p) -> p m", p=P))
        nc.sync.dma_start(out=pb_sb, in_=pb_d.rearrange("(m p) -> p m", p=P))
        nc.sync.dma_start(out=f1b_sb, in_=f1b_d.rearrange("(m p) -> p m", p=P))
        nc.sync.dma_start(out=f2b_sb, in_=f2b_d.rearrange("(m p) -> p m", p=P))
        nc.scalar.dma_start(out=pw_sb, in_=pw_d.rearrange("(kc p) o -> p kc o", p=P))
        nc.scalar.dma_start(out=f1w_sb, in_=f1w_d.rearrange("(kc p) o -> p kc o", p=P))
        nc.scalar.dma_start(out=f2w_sb, in_=f2w_d.rearrange("(kc p) o -> p kc o", p=P))

        # ============ Phase B0: v projection + qk projections (DoubleRow) ============
        with tc.tile_pool(name="psV", space="PSUM", bufs=2) as psV:
            nc.vector.memset(vext[:, :, :, HD:HD + 1], 1.0)
            for t in range(NT):
                pv = psV.tile([P, 2, 512], F32, tag="pv", name=f"pv_{t}")
                for half, (c0, cw) in enumerate(((0, 512), (512, 256))):
                    for ep in range(3):
                        nc.tensor.matmul(
                            pv[:, half, :cw],
                            actT[:, 2 * ep:2 * ep + 2, t * P:(t + 1) * P],
                            vw_sb[:, 2 * ep:2 * ep + 2, c0:c0 + cw],
                            start=(ep == 0), stop=(ep == 2),
                            perf_mode=DR,
                        )
                nc.vector.tensor_scalar(
                    out=vext[:, t, 0:8, 0:HD],
                    in0=pv[:, 0, :].rearrange("p (h d) -> p h d", d=HD),
                    scalar1=IWS, scalar2=None, op0=ALU.mult,
                )
                nc.vector.tensor_scalar(
                    out=vext[:, t, 8:12, 0:HD],
                    in0=pv[:, 1, 0:256].rearrange("p (h d) -> p h d", d=HD),
                    scalar1=IWS, scalar2=None, op0=ALU.mult,
                )
            for j in range(2 * EC):
                pqk = psV.tile([P, 2, 512], F32, tag="pv", name=f"pqk_{j}")
                for n in range(NC2):
                    for ep in range(3):
                        nc.tensor.matmul(
                            pqk[:, n, :],
                            qkw_sb[:, 2 * ep:2 * ep + 2, j * P:(j + 1) * P],
                            actT[:, 2 * ep:2 * ep + 2, n * 512:(n + 1) * 512],
                            start=(ep == 0), stop=(ep == 2),
                            perf_mode=DR,
                        )
                nc.scalar.activation(
                    out=qkT[:, j, :].rearrange("p (n c) -> p n c", c=512),
                    in_=pqk,
                    func=AF.Identity, bias=qkb_sb[:, j:j + 1], scale=IWS,
                )

        # ================= Phase B1: attention, one head-pair at a time =================
        # S: 64x128 row-tiled bf16 matmul pairs (head0 on partitions 0-63 / tile
        # (0,0), head1 on 64-127 / tile (64,0)), emitted back-to-back so they run
        # concurrently in the PE array. exp split: sub0 -> ScalarE exact exp,
        # sub1 -> VectorE fp8-bit synthesis. PV: fp8 DoubleRow over mt-tile pairs
        # with the ones column producing the softmax denominator in row 64.
        with tc.tile_pool(name="psS", space="PSUM", bufs=2) as psS, \
             tc.tile_pool(name="psPO", space="PSUM", bufs=4) as psPO:
            for hp in range(HP):
                pps = {}
                for sub in range(2):
                    for mp in range(4):
                        pps[(sub, mp)] = work.tile(
                            [P, 2, NC2, 512], F8, tag="pp", bufs=8,
                            name=f"pp_h{hp}s{sub}_m{mp}",
                        )
                pos = {}
                for mt in range(NT):
                    sS = {}
                    for sub in range(2):
                        sS[sub] = psS.tile([P, NC2, 512], F32, tag="s",
                                           name=f"s_{hp}_{sub}_{mt}")
                    for n in range(NC2):
                        for sub in range(2):
                            base = sub * HD
                            nc.tensor.matmul(
                                sS[sub][:, n, :],
                                qkT[base:base + HD, EC + hp, mt * P:(mt + 1) * P],
                                qkT[base:base + HD, hp, n * 512:(n + 1) * 512],
                                start=True, stop=True,
                            )
                    # sub0: exact exp on ScalarE; sub1: fp8-bit exp on VectorE
                    nc.scalar.activation(
                        out=pps[(0, mt // 2)][:, mt % 2, :, :],
                        in_=sS[0], func=AF.Exp, scale=SCALE, bias=negone,
                    )
                    nc.vector.tensor_scalar(
                        out=pps[(1, mt // 2)][:, mt % 2, :, :].bitcast(I8),
                        in0=sS[1], scalar1=EA, scalar2=EB,
                        op0=ALU.mult, op1=ALU.add,
                    )
                    # interleave PV of the completed mt pair to keep PE dense
                    if mt % 2 == 1:
                        mp = mt // 2
                        for sub in range(2):
                            h = 2 * hp + sub
                            for n in range(NC2):
                                po = pos.get((sub, n))
                                if po is None:
                                    po = psPO.tile([P, 512], F32, tag="po",
                                                   name=f"po_{hp}_{sub}_{n}")
                                    pos[(sub, n)] = po
                                nc.tensor.matmul(
                                    po[0:HD + 1, :],
                                    vext[:, 2 * mp:2 * mp + 2, h, 0:HD + 1],
                                    pps[(sub, mp)][:, :, n, :],
                                    start=(mp == 0), stop=(mp == 3),
                                    perf_mode=DR,
                                )
                # normalize: denom rows straight out of PSUM via DMA, oT
                # multiply reads PSUM directly (no intermediate copy)
                dpack = stat_pool.tile([4, 512], F32, tag="dpack", bufs=2,
                                       name=f"dp_{hp}")
                for sub in range(2):
                    for n in range(NC2):
                        idx = sub * NC2 + n
                        drow = work.tile([HD + 1, 512], F32, tag="drow", bufs=4,
                                         name=f"dr_{hp}_{idx}")
                        nc.vector.tensor_copy(out=drow[HD:HD + 1, :],
                                              in_=pos[(sub, n)][HD:HD + 1, :])
                        nc.sync.dma_start(out=dpack[idx:idx + 1, :],
                                          in_=drow[HD:HD + 1, :])
                rpack = stat_pool.tile([4, 512], F32, tag="rpack", bufs=2,
                                       name=f"rp_{hp}")
                nc.vector.reciprocal(out=rpack, in_=dpack)
                for sub in range(2):
                    doff = sub * HD
                    for n in range(NC2):
                        idx = sub * NC2 + n
                        rtmp = stat_pool.tile([1, 512], F32, tag="rtmp", bufs=2,
                                              name=f"rt_{hp}_{idx}")
                        nc.sync.dma_start(out=rtmp, in_=rpack[idx:idx + 1, :])
                        rb = work.tile([HD, 512], F32, tag="rb", bufs=4,
                                       name=f"rb_{hp}_{idx}")
                        nc.gpsimd.partition_broadcast(rb, rtmp)
                        nc.vector.tensor_tensor(
                            out=oT[doff:doff + HD, hp, n * 512:(n + 1) * 512],
                            in0=pos[(sub, n)][0:HD, :], in1=rb, op=ALU.mult,
                        )

        # ====== Phase C: proj + residual + LN2 ======
        # All proj matmuls (both n-chunks) first; transposes and the LN2 DVE
        # chains are ordered so the PE always has transposes to run while the
        # DVE works through stats/apply.
        with tc.tile_pool(name="psC", space="PSUM", bufs=1) as psC:
            mv2s = {}
            for n in range(NC2):
                prns = []
                for me in range(EC):
                    ppr = psC.tile([P, 512], F32, tag="ppr", bufs=6,
                                   name=f"ppr_{me}_{n}")
                    for kp in range(3):
                        nc.tensor.matmul(
                            ppr,
                            pw_sb[:, 2 * kp:2 * kp + 2, me * P:(me + 1) * P],
                            oT[:, 2 * kp:2 * kp + 2, n * 512:(n + 1) * 512],
                            start=(kp == 0), stop=(kp == 2),
                            perf_mode=DR,
                        )
                    prn = work.tile([P, 512], BF16, tag="prn", bufs=8,
                                    name=f"prn_{me}_{n}")
                    nc.scalar.activation(
                        out=prn, in_=ppr, func=AF.Identity,
                        bias=pb_sb[:, me:me + 1], scale=IWS,
                    )
                    prns.append(prn)
                for me in range(EC):
                    ptr = psC.tile([P, 4, P], BF16, tag="tr", bufs=2,
                                   name=f"trp_{me}_{n}")
                    for j in range(4):
                        nc.tensor.transpose(ptr[:, j, :],
                                            prns[me][:, j * P:(j + 1) * P], ident)
                    nc.vector.tensor_tensor(
                        out=x1[:, 4 * n:4 * n + 4, me * P:(me + 1) * P],
                        in0=x1[:, 4 * n:4 * n + 4, me * P:(me + 1) * P],
                        in1=ptr, op=ALU.add,
                    )
                mv2 = stat_pool.tile([P, 2, 4], F32, tag="mv2", bufs=2, name=f"mv2_{n}")
                for j in range(4):
                    stats = stat_pool.tile([P, 3, 6], F32, tag="stats",
                                           name=f"st2_{4*n+j}")
                    _ln_stats(nc, x1[:, 4 * n + j, :], mv2[:, :, j], stats)
                nc.scalar.activation(
                    out=mv2[:, 1, :], in_=mv2[:, 1, :], func=AF.Sqrt,
                    bias=eps_t, scale=1.0,
                )
                nc.vector.reciprocal(out=mv2[:, 1, :], in_=mv2[:, 1, :])
                mv2s[n] = mv2
            for n in range(NC2):
                for j in range(4):
                    t = 4 * n + j
                    h_t = work.tile([P, EMB], BF16, tag="h", bufs=4, name=f"h2_{t}")
                    nc.vector.tensor_scalar(
                        out=h_t, in0=x1[:, t, :],
                        scalar1=mv2s[n][:, 0:1, j], scalar2=mv2s[n][:, 1:2, j],
                        op0=ALU.subtract, op1=ALU.mult,
                    )
                    _transpose_to_featmajor(nc, psC, h_t, actT, t, ident)
        h2T = actT

        # ================= Phase F: MLP + residual + output =================
        out_r = out_d.rearrange("(t p) e -> p t e", p=P)
        with tc.tile_pool(name="psF", space="PSUM", bufs=1) as psF:
            for n in range(NC2):
                acc = [
                    psF.tile([P, 2, 512], F32, tag=f"acc{i}", bufs=1, name=f"acc_{n}_{i}")
                    for i in range(3)
                ]

                def acc_sl(e):
                    return acc[e // 2][:, e % 2, :]

                for hcp in range(HC // 2):
                    apair = work.tile([P, 2, 512], F8, tag="apair", bufs=3,
                                      name=f"a_{n}_{hcp}")
                    for u in range(2):
                        hc = 2 * hcp + u
                        pf1 = psF.tile([P, 512], F32, tag="f1", bufs=2,
                                       name=f"pf1_{n}_{hc}")
                        for ep in range(3):
                            nc.tensor.matmul(
                                pf1,
                                f1w_sb[:, 2 * ep:2 * ep + 2, hc * P:(hc + 1) * P],
                                h2T[:, 2 * ep:2 * ep + 2, n * 512:(n + 1) * 512],
                                start=(ep == 0), stop=(ep == 2),
                                perf_mode=DR,
                            )
                        nc.scalar.activation(
                            out=apair[:, u, :], in_=pf1, func=AF.Gelu,
                            bias=f1b_sb[:, hc:hc + 1], scale=IWS,
                        )
                    for e in range(EC):
                        nc.tensor.matmul(
                            acc_sl(e),
                            f2w_sb[:, 2 * hcp:2 * hcp + 2, e * P:(e + 1) * P],
                            apair,
                            start=(hcp == 0), stop=(hcp == HC // 2 - 1),
                            perf_mode=DR,
                        )
                fout = work.tile([P, 4, EMB], BF16, tag="fout", bufs=2, name=f"fo_{n}")
                for e in range(EC):
                    fr = work.tile([P, 512], BF16, tag="prn", bufs=8, name=f"fr_{n}_{e}")
                    nc.scalar.activation(
                        out=fr, in_=acc_sl(e), func=AF.Identity,
                        bias=f2b_sb[:, e:e + 1], scale=IWS,
                    )
                    ptr = psF.tile([P, 4, P], BF16, tag="f1", bufs=2, name=f"trf_{n}_{e}")
                    for j in range(4):
                        nc.tensor.transpose(ptr[:, j, :], fr[:, j * P:(j + 1) * P], ident)
                    nc.vector.tensor_tensor(
                        out=fout[:, :, e * P:(e + 1) * P],
                        in0=x1[:, 4 * n:4 * n + 4, e * P:(e + 1) * P],
                        in1=ptr, op=ALU.add,
                    )
                for j in range(4):
                    t = 4 * n + j
                    nc.sync.dma_start(out=out_r[:, t, :], in_=fout[:, j, :])


def fold_inputs(inputs):
    """Fold LN gamma/beta and v-bias into downstream weights; scale weights by
    WS=64 and quantize to fp8 e4m3 (IEEE, max 240 — matches TRN FP8_EXP4)."""
    import ml_dtypes

    f = {k: np.asarray(v, dtype=np.float32) for k, v in inputs.items()}
    qkw = f["ln1_g"][:, None] * f["qk_w"]
    qkb = f["ln1_b"] @ f["qk_w"]
    vw = f["ln1_g"][:, None] * f["v_w"]
    vb = f["ln1_b"] @ f["v_w"]
    # softmax rows sum to 1 => o = attn @ (v + 1 vb^T) = attn@v + vb
    pb = f["proj_b"] + vb @ f["proj_w"]
    f1w = f["ln2_g"][:, None] * f["fc1_w"]
    f1b = f["fc1_b"] + f["ln2_b"] @ f["fc1_w"]

    fp8 = ml_dtypes.float8_e4m3

    def w8(a):
        return np.ascontiguousarray(np.clip(a * WS, -240.0, 240.0).astype(fp8))

    return {
        "qkw": w8(qkw),
        "qkb": np.ascontiguousarray(qkb),
        "vw": w8(vw),
        "pw": w8(f["proj_w"]),
        "pb": np.ascontiguousarray(pb),
        "f1w": w8(f1w),
        "f1b": np.ascontiguousarray(f1b),
        "f2w": w8(f["fc2_w"]),
        "f2b": np.ascontiguousarray(f["fc2_b"]),
    }


def prep_core_inputs(inputs):
    """Full-input dict -> list of per-core in_maps (x cast to bf16)."""
    import ml_dtypes

    x = np.asarray(inputs["x"], dtype=np.float32).astype(ml_dtypes.bfloat16)
    folded = fold_inputs({k: v for k, v in inputs.items() if k != "x"})
    return [
        {"x": np.ascontiguousarray(x[c]), **folded} for c in range(_N_CORES)
    ]


_INPUT_SHAPES = {
    "x": (SEQ, EMB),
    "qkw": (EMB, 2 * EMB),
    "qkb": (2 * EMB,),
    "vw": (EMB, EMB),
    "pw": (EMB, EMB),
    "pb": (EMB,),
    "f1w": (EMB, MLPD),
    "f1b": (MLPD,),
    "f2w": (MLPD, EMB),
    "f2b": (EMB,),
}

_F8_INPUTS = {"qkw", "vw", "pw", "f1w", "f2w"}
_BF16_INPUTS = {"x"}

_N_CORES = 8
_compiled = {}


def _build_nc(num_devices=_N_CORES):
    import concourse.tile as tile
    from concourse import bacc

    nc = bacc.Bacc(
        "TRN2", target_bir_lowering=False, debug=False, num_devices=num_devices
    )

    def dt_of(name):
        if name in _F8_INPUTS:
            return F8
        if name in _BF16_INPUTS:
            return BF16
        return F32

    ins = {
        name: nc.dram_tensor(
            name, list(shape), dt_of(name), kind="ExternalInput"
        ).ap()
        for name, shape in _INPUT_SHAPES.items()
    }
    out = nc.dram_tensor("out", [SEQ, EMB], BF16, kind="ExternalOutput").ap()
    with tile.TileContext(nc) as tc:
        build_block(tc, {"out": out}, ins)
    nc.compile()
    return nc


def kernel(**inputs):
    """Full-input entry point: x [8, 1024, 768] + weights -> [8, 1024, 768]."""
    from concourse.bass_utils import run_bass_kernel_spmd

    if "nc" not in _compiled:
        _compiled["nc"] = _build_nc()
    nc = _compiled["nc"]

    in_maps = prep_core_inputs(inputs)
    res = run_bass_kernel_spmd(nc, in_maps, core_ids=list(range(_N_CORES)))
    return np.stack([res.results[c]["out"] for c in range(_N_CORES)]).astype(
        np.float32
    )
